# revision 10
# baseline (speedup 1.0000x reference)
"""GCN layer (SpMM segment-sum + dense projection + relu) on 8 TRN2 NeuronCores.

    AH[i] = sum_{e: src[e]==i} val[e] * H[dst[e]];  out = relu(AH @ W + b)

Strategy (src-sharded, one SPMD program on 8 cores):
  - Each core owns 12500 contiguous src rows of the output. The host buckets
    edges by owner core, sorts by src, and packs per-src segments into
    "windows" of <=128 srcs whose edges are split by dst-chunk (4 chunks of
    25000 H rows, int16-indexable) with <=512 edges per chunk -> each window
    is exactly 16 batches of 128 edge slots (4 per chunk, sentinel-padded).
  - H is staged as fp16 (the SpMM matmul already ran in fp16; RNE cast on
    host == the previous on-device ActivationE cast, bit-identical output),
    halving gather bytes and removing the per-block cast.
  - Per gather block (4 windows) the core issues two 1024-idx dma_gather
    calls per chunk rotating SWDGE queues {0,1,2,3} (256B descriptors; the
    gather is bound by the SWDGE per-descriptor path at ~4.5ns/desc, so
    queue pattern and call size are tuned empirically).
  - Per 128-edge batch, VectorE builds P[e, slot] = (iota==srclocal[e])*val[e]
    (one fused tensor_scalar in DVE 4x perf mode), and TensorE accumulates
    AH^T[feat, slot] += G_batch^T @ P in PSUM (fp16 operands, fp32 accum).
  - Per window: AH^T -> fp32 projection matmul with W -> +b, relu -> rows of a
    per-core virtual-slot output; the host permutes virtual rows back to
    global node order (pure indexing).

The host only reorders indices/values, casts H to fp16, and permutes output
rows (sharding prep); all SpMM data movement and FLOPs happen on device.
"""
import numpy as np

import concourse.bacc as bacc
import concourse.bass as bass
import concourse.mybir as mybir
import concourse.tile as tile

import time
import jax
from jax.sharding import Mesh, PartitionSpec, NamedSharding
from jax.experimental.shard_map import shard_map

import concourse.bass as bass
import concourse.mybir as mybir
from concourse import bass2jax
from concourse.bass2jax import _bass_exec_p, install_neuronx_cc_hook


def _collect_io(nc, skip_names=()):
    in_names, out_names, out_avals = [], [], []
    for alloc in nc.m.functions[0].allocations:
        if not isinstance(alloc, mybir.MemoryLocationSet):
            continue
        name = alloc.memorylocations[0].name
        if alloc.kind == "ExternalInput":
            if name in skip_names:
                continue
            in_names.append(name)
        elif alloc.kind == "ExternalOutput":
            out_names.append(name)
            out_avals.append(
                jax.core.ShapedArray(tuple(alloc.tensor_shape), mybir.dt.np(alloc.dtype))
            )
    return in_names, out_names, out_avals


class SpmdRunner:
    def __init__(self, nc: bass.Bass, n_cores: int = 8):
        install_neuronx_cc_hook()
        self.nc = nc
        self.n_cores = n_cores
        self.partition_name = (
            nc.partition_id_tensor.name if nc.partition_id_tensor else None
        )
        self.in_names, self.out_names, self.out_avals = _collect_io(
            nc, skip_names=(self.partition_name,) if self.partition_name else ()
        )
        devices = jax.devices()[:n_cores]
        assert len(devices) == n_cores
        self.mesh = Mesh(np.asarray(devices), ("core",))
        self.sharding = NamedSharding(self.mesh, PartitionSpec("core"))
        self._dev_args = None
        self._jitted = None

    def _make_jit(self):
        in_names = self.in_names
        out_names, out_avals = self.out_names, self.out_avals
        n_params = len(in_names)
        all_in_names = tuple(in_names) + tuple(out_names)
        partition_name = self.partition_name
        if partition_name is not None:
            all_in_names = all_in_names + (partition_name,)

        def _body(*args):
            extra = [bass2jax.partition_id_tensor()] if partition_name else []
            outs = _bass_exec_p.bind(
                *args, *extra,
                out_avals=tuple(out_avals),
                in_names=all_in_names,
                out_names=tuple(out_names),
                lowering_input_output_aliases=(),
                sim_require_finite=True,
                sim_require_nnan=True,
                nc=self.nc,
            )
            return tuple(outs)

        n_outs = len(out_avals)
        in_specs = (PartitionSpec("core"),) * (n_params + n_outs)
        out_specs = (PartitionSpec("core"),) * n_outs
        return jax.jit(
            shard_map(_body, mesh=self.mesh, in_specs=in_specs,
                      out_specs=out_specs, check_rep=False),
            keep_unused=True,
        )

    def prepare(self, in_maps):
        """device_put concatenated inputs once; returns nothing."""
        n = self.n_cores
        assert len(in_maps) == n
        concat_in = [
            np.concatenate([np.asarray(in_maps[c][name]) for c in range(n)], axis=0)
            for name in self.in_names
        ]
        concat_zeros = [
            np.zeros((n * a.shape[0], *a.shape[1:]), a.dtype) for a in self.out_avals
        ]
        self._dev_args = [jax.device_put(a, self.sharding)
                          for a in concat_in + concat_zeros]
        if self._jitted is None:
            self._jitted = self._make_jit()

    def run(self):
        """One execution; returns per-core result dicts."""
        assert self._dev_args is not None, "call prepare() first"
        out_arrs = self._jitted(*self._dev_args)
        out_arrs = [np.asarray(a) for a in out_arrs]
        return [
            {name: out_arrs[i].reshape(self.n_cores, *self.out_avals[i].shape)[c]
             for i, name in enumerate(self.out_names)}
            for c in range(self.n_cores)
        ]

    def _timed_burst(self, n):
        t0 = time.time()
        o = None
        for _ in range(n):
            o = self._jitted(*self._dev_args)
        jax.block_until_ready(o)
        return time.time() - t0

    def measure_exec_time(self, n_lo=3, n_hi=43, trials=5, repeats=3):
        """Marginal per-execution wall seconds (pure HW exec, RPC amortized).
        Median over `repeats` marginal estimates for noise robustness."""
        assert self._dev_args is not None, "call prepare() first"
        self._timed_burst(2)  # warm
        ests = []
        for _ in range(repeats):
            lo = min(self._timed_burst(n_lo) for _ in range(trials))
            hi = min(self._timed_burst(n_hi) for _ in range(trials))
            ests.append((hi - lo) / (n_hi - n_lo))
        ests.sort()
        return ests[len(ests) // 2]

N_NODES = 100000
F = 128          # feature dim == num units
N_CORES = 8
RPC = N_NODES // N_CORES   # src rows per core (12500)
NCHUNK = 4
CHUNK = 25000    # H rows per gather chunk (int16-indexable)
QB = 4           # batches per (window, chunk)
NB = NCHUNK * QB           # 16 batches (2048 edge slots) per window
BW = 4           # windows per gather block
CB = NB * BW     # 64 batch columns per block
CALL_IDX = BW * QB * 128   # 2048 idxs per dma_gather call
CDT = mybir.dt.float16     # compute dtype for the SpMM accumulation
NP_CDT = np.float16


# ----------------------------------------------------------------- host prep

def _pack_core(chunk_counts):
    """Pack srcs into windows obeying <=128 srcs and <=QB*128 edges per
    chunk. chunk_counts: [RPC, 4] per-src per-chunk edge counts. Returns
    list of windows (sorted lists of src ids).

    Next-fit-with-skip (greedy forward scan per window) + random restarts:
    reaches 100 windows/core on this data vs 104 for plain next-fit and
    112 for first-fit-decreasing (the per-chunk quotas punish degree-sorted
    packing). 100 matters: W_win pads to a multiple of BW=4, so <=100
    windows -> 25 gather blocks instead of 26 (~4% fewer descriptors)."""
    cap = QB * 128

    def nf_skip(order):
        remaining = list(order)
        windows = []
        while remaining:
            cur, used, skipped = [], np.zeros(NCHUNK, np.int64), []
            for idx, s in enumerate(remaining):
                if len(cur) < 128 and np.all(used + chunk_counts[s] <= cap):
                    cur.append(int(s))
                    used += chunk_counts[s]
                    if len(cur) == 128:
                        skipped.extend(remaining[idx + 1:])
                        break
                else:
                    skipped.append(s)
            windows.append(sorted(cur))
            remaining = skipped
        return windows

    best = None
    for seed in range(8):
        rng = np.random.default_rng(seed)
        order = np.arange(RPC) if seed == 0 else rng.permutation(RPC)
        w = nf_skip(order)
        if best is None or len(w) < len(best):
            best = w
        if len(best) <= 100:
            break
    return best


def prepare(H, edge_vals, W, b, edge_src, edge_dst):
    H = np.ascontiguousarray(np.asarray(H, np.float32).astype(NP_CDT))
    edge_vals = np.asarray(edge_vals, np.float32)
    W = np.ascontiguousarray(np.asarray(W, np.float32))
    b = np.asarray(b, np.float32)
    edge_src = np.asarray(edge_src, np.int64)
    edge_dst = np.asarray(edge_dst, np.int64)

    per_core = []
    for c in range(N_CORES):
        sel = (edge_src // RPC) == c
        s = edge_src[sel] - c * RPC
        d = edge_dst[sel]
        v = edge_vals[sel]
        k = d // CHUNK
        # sort by (src, chunk) so each (src, chunk) run is contiguous
        order = np.lexsort((k, s))
        s, d, v, k = s[order], d[order], v[order], k[order]
        # per-src per-chunk counts
        cc = np.zeros((RPC, NCHUNK), np.int64)
        np.add.at(cc, (s, k), 1)
        windows = _pack_core(cc)
        per_core.append((s, d, v, k, cc, windows))

    W_win = max(len(pc[5]) for pc in per_core)
    W_win = -(-W_win // BW) * BW
    nblk = W_win // BW

    # per-core slot arrays in canonical order [window, chunk, q, j]
    nslots = W_win * NB * 128
    lidx = np.zeros((N_CORES, W_win, NCHUNK, QB * 128), np.int16)
    srcl = np.zeros((N_CORES, W_win, NB, 128), np.float32)
    vals = np.zeros((N_CORES, W_win, NB, 128), np.float32)
    rowmap = np.full((N_CORES, W_win * 128), -1, np.int64)

    for c in range(N_CORES):
        s, d, v, k, cc, windows = per_core[c]
        # start offset of each (src, chunk) run in the sorted edge list
        flat_counts = cc.reshape(-1)
        starts = np.zeros(RPC * NCHUNK + 1, np.int64)
        np.cumsum(flat_counts, out=starts[1:])
        for w, wsrcs in enumerate(windows):
            fill = np.zeros(NCHUNK, np.int64)
            for j, sid in enumerate(wsrcs):
                rowmap[c, w * 128 + j] = c * RPC + sid
                for ck in range(NCHUNK):
                    n = cc[sid, ck]
                    if n == 0:
                        continue
                    d0 = starts[sid * NCHUNK + ck]
                    pos = fill[ck]
                    lidx[c, w, ck, pos:pos + n] = (d[d0:d0 + n] - ck * CHUNK)
                    q0, j0 = divmod(pos, 128)
                    # slots within chunk are linear; srcl/vals are [NB, 128]
                    flat = ck * QB * 128 + pos
                    srcl.reshape(N_CORES, W_win, -1)[c, w, flat:flat + n] = j
                    vals.reshape(N_CORES, W_win, -1)[c, w, flat:flat + n] = v[d0:d0 + n]
                    fill[ck] += n

    # gather idx arrays: per (block, chunk) call of 2048 idxs.
    # call flat position i = (w_loc*QB + q)*128 + j  <-> lidx[c, blk*BW+w_loc, ck, q*128+j]
    # wrapped int16 [16, 128] replicated to [128, 128] per call.
    gidx = np.zeros((N_CORES, nblk, NCHUNK, 128, CALL_IDX // 16), np.int16)
    l5 = lidx.reshape(N_CORES, nblk, BW, NCHUNK, QB * 128)
    for c in range(N_CORES):
        for blk in range(nblk):
            for ck in range(NCHUNK):
                flat = l5[c, blk, :, ck, :].reshape(-1)  # [BW*QB*128] in (w_loc, q, j)
                wrapped = flat.reshape(CALL_IDX // 16, 16).T  # [16, 128]
                gidx[c, blk, ck] = np.tile(wrapped, (8, 1))

    # srcl/val device layout [nblk, 128, CB]: column cb = w_loc*NB + (ck*QB+q)
    def to_cols(a):
        # a: [N_CORES, W_win, NB, 128] -> [N_CORES, nblk, 128, BW*NB]
        return (a.reshape(N_CORES, nblk, BW, NB, 128)
                 .transpose(0, 1, 4, 2, 3)
                 .reshape(N_CORES, nblk, 128, CB))

    srcl = to_cols(srcl)
    vals = to_cols(vals)

    iota = np.tile(np.arange(128, dtype=NP_CDT), (128, 1))
    brep = np.tile(b, (128, 1)).astype(np.float32)

    in_maps = []
    for c in range(N_CORES):
        in_maps.append({
            "H": H,
            "gidx": gidx[c],
            "srcl": srcl[c],
            "val": vals[c],
            "iota": iota,
            "Wm": W,
            "brep": brep,
        })
    return in_maps, rowmap, W_win, nblk


# ------------------------------------------------------------- device program

G_SPLIT = 1024          # idxs per dma_gather call
G_QUEUES = (0, 1, 2, 3)       # SWDGE queue rotation across gather calls


def build_program(nblk, repeat=1):
    nc = bacc.Bacc("TRN2", target_bir_lowering=False, debug=False,
                   num_swdge_queues=4)
    H_t = nc.dram_tensor("H", [N_NODES, F], CDT, kind="ExternalInput")
    gidx_t = nc.dram_tensor("gidx", [nblk, NCHUNK, 128, CALL_IDX // 16],
                            mybir.dt.int16, kind="ExternalInput")
    srcl_t = nc.dram_tensor("srcl", [nblk, 128, CB], mybir.dt.float32, kind="ExternalInput")
    val_t = nc.dram_tensor("val", [nblk, 128, CB], mybir.dt.float32, kind="ExternalInput")
    iota_t = nc.dram_tensor("iota", [128, 128], CDT, kind="ExternalInput")
    Wm_t = nc.dram_tensor("Wm", [F, F], mybir.dt.float32, kind="ExternalInput")
    brep_t = nc.dram_tensor("brep", [128, F], mybir.dt.float32, kind="ExternalInput")
    out_t = nc.dram_tensor("outv", [nblk * BW * 128, F], mybir.dt.float32,
                           kind="ExternalOutput")

    nsub = CALL_IDX // G_SPLIT
    wcols = G_SPLIT // 16
    call_i = 0

    with tile.TileContext(nc) as tc:
        with (
            tc.tile_pool(name="consts", bufs=1) as cpool,
            tc.tile_pool(name="gpool", bufs=3) as gpool,
            tc.tile_pool(name="inpool", bufs=3) as inpool,
            tc.tile_pool(name="ppool", bufs=4) as ppool,
            tc.tile_pool(name="phase2", bufs=3) as p2pool,
            tc.tile_pool(name="opool", bufs=4) as opool,
            tc.tile_pool(name="psacc", bufs=5, space="PSUM") as psacc,
            tc.tile_pool(name="psout", bufs=3, space="PSUM") as psout,
        ):
            iota_sb = cpool.tile([128, 128], CDT)
            nc.sync.dma_start(iota_sb[:], iota_t[:])
            Wm_sb = cpool.tile([F, F], mybir.dt.float32)
            nc.sync.dma_start(Wm_sb[:], Wm_t[:])
            brep_sb = cpool.tile([128, F], mybir.dt.float32)
            nc.sync.dma_start(brep_sb[:], brep_t[:])

            for blk in [b for _ in range(repeat) for b in range(nblk)]:
                srcl_sb = inpool.tile([128, CB], mybir.dt.float32, tag="srcl")
                nc.sync.dma_start(srcl_sb[:], srcl_t[blk])
                val_sb = inpool.tile([128, CB], mybir.dt.float32, tag="val")
                nc.sync.dma_start(val_sb[:], val_t[blk])

                G = gpool.tile([128, NCHUNK, BW * QB, F], CDT)
                for ck in range(NCHUNK):
                    gidx_sb = inpool.tile([128, CALL_IDX // 16], mybir.dt.int16,
                                          tag="gidx")
                    nc.sync.dma_start(gidx_sb[:], gidx_t[blk, ck])
                    for s in range(nsub):
                        q = G_QUEUES[call_i % len(G_QUEUES)]
                        call_i += 1
                        nc.gpsimd.dma_gather(
                            out_ap=G[:, ck, s * (G_SPLIT // 128):
                                     (s + 1) * (G_SPLIT // 128)],
                            in_ap=H_t[ck * CHUNK:(ck + 1) * CHUNK, :],
                            idxs_ap=gidx_sb[:, s * wcols:(s + 1) * wcols],
                            num_idxs=G_SPLIT,
                            num_idxs_reg=G_SPLIT,
                            elem_size=F,
                            single_packet=False,
                            queue_num=q,
                        )

                for wl in range(BW):
                    ps = psacc.tile([128, 128], mybir.dt.float32, space="PSUM")
                    for ck in range(NCHUNK):
                        for q in range(QB):
                            cb = wl * NB + ck * QB + q
                            P = ppool.tile([128, 128], CDT)
                            nc.vector.tensor_scalar(
                                out=P[:],
                                in0=iota_sb[:],
                                scalar1=srcl_sb[:, cb:cb + 1],
                                scalar2=val_sb[:, cb:cb + 1],
                                op0=mybir.AluOpType.is_equal,
                                op1=mybir.AluOpType.mult,
                            )
                            # psum[f, slot] += G_batch^T @ P
                            nc.tensor.matmul(
                                ps[:],
                                lhsT=G[:, ck, wl * QB + q, :],
                                rhs=P[:],
                                start=(ck == 0 and q == 0),
                                stop=(ck == NCHUNK - 1 and q == QB - 1),
                            )
                    # ---- fused phase 2 for this window ----
                    ahT_sb = p2pool.tile([128, 128], mybir.dt.float32, tag="ahT")
                    nc.vector.tensor_copy(ahT_sb[:], ps[:])
                    o_ps = psout.tile([128, F], mybir.dt.float32, space="PSUM")
                    nc.tensor.matmul(o_ps[:], lhsT=ahT_sb[:], rhs=Wm_sb[:],
                                     start=True, stop=True)
                    o_sb = opool.tile([128, F], mybir.dt.float32)
                    nc.vector.tensor_tensor(out=o_sb[:], in0=o_ps[:],
                                            in1=brep_sb[:],
                                            op=mybir.AluOpType.add)
                    nc.vector.tensor_scalar(out=o_sb[:], in0=o_sb[:],
                                            scalar1=0.0, scalar2=None,
                                            op0=mybir.AluOpType.max)
                    w = blk * BW + wl
                    nc.sync.dma_start(out_t[w * 128:(w + 1) * 128, :], o_sb[:])
    nc.compile()
    return nc


# ------------------------------------------------------------------ interface

_CACHE = {}


def _get_runner(nblk, repeat=1):
    key = (nblk, repeat)
    if key not in _CACHE:
        _CACHE[key] = SpmdRunner(build_program(nblk, repeat), N_CORES)
    return _CACHE[key]


def kernel(H, edge_vals, W, b, edge_src, edge_dst):
    in_maps, rowmap, W_win, nblk = prepare(H, edge_vals, W, b, edge_src, edge_dst)
    runner = _get_runner(nblk)
    runner.prepare(in_maps)
    results = runner.run()
    out = np.zeros((N_NODES, F), np.float32)
    for c in range(N_CORES):
        rm = rowmap[c]
        valid = rm >= 0
        out[rm[valid]] = results[c]["outv"][valid]
    return out

